# revision 6
# baseline (speedup 1.0000x reference)
"""Multi-head attention kernel for TRN2, 8 NeuronCores.

Problem: x (8, 256, 32, 32); qkv = w_qkv @ x_flat per batch; q, k l2-normalized
over the token axis; sim = 10 * q^T k; softmax over keys; out = attn @ v^T;
y = w_out @ out_hidden + b_out.

Sharding: pure data-parallel — batch 8 across 8 cores, one batch each.
No collectives. Weights replicated (transposed host-side to feed the PE
stationary operand directly).

Layout choices (per core, one batch):
  - qk proj computed as (o=1024, n=1024): lhsT = w_qkv[:1024].T chunks,
    rhs = x chunks. l2 norm over n = free-axis reduce (per-partition).
  - v projected TRANSPOSED: vT (n=1024, hid=512) via lhsT = x, rhs = w_v.T.
    Stored interleaved with a ones column per head -> [v_h | 1] (65 cols/head)
    so the attention AV matmul's 65th output row is the softmax denominator.
  - S^T = k_h^T q_h per head: (j=1024, i=1024), softmax over PARTITION axis j:
    values are bounded (|S| < 1), so exp needs no max subtraction; denominator
    comes from the ones row. exp runs on ScalarE straight out of PSUM.
  - U = [v|1] @ expS^T accumulated over j chunks in PSUM; normalization:
    recip of row 64 (DVE approx), broadcast over 64 partitions via a K=1
    matmul, elementwise mul into out_hidden.
  - y = w_out.T-chunks @ out_hidden + b_out, DMA out.

All matmuls run float32r (f32 storage, 1 cycle/row on PE at N>=256).
"""

import numpy as np

import concourse.bass as bass
import concourse.mybir as mybir
import concourse.tile as tile
from concourse import bacc
from concourse.bass_utils import run_bass_kernel_spmd

F32 = mybir.dt.float32
F32R = mybir.dt.float32r
AF = mybir.ActivationFunctionType

B = 8          # batch (one per core)
C = 256        # input channels
N = 1024       # tokens (32*32)
HID = 512      # heads * dim_head
HEADS = 8
DH = 64
NCORES = 8

_cache = {}


def _build():
    nc = bacc.Bacc("TRN2", target_bir_lowering=False, debug=False)

    x_d = nc.dram_tensor("x", [C, N], F32R, kind="ExternalInput")
    wqk_d = nc.dram_tensor("w_qkT", [C, 2 * HID], F32R, kind="ExternalInput")
    wv_d = nc.dram_tensor("w_vT", [C, HID], F32R, kind="ExternalInput")
    wout_d = nc.dram_tensor("w_outT", [HID, C], F32R, kind="ExternalInput")
    b_d = nc.dram_tensor("b_out", [C, 1], F32, kind="ExternalInput")
    out_d = nc.dram_tensor("out", [C, N], F32, kind="ExternalOutput")

    with tile.TileContext(nc) as tc:
        _body(nc, tc, x_d, wqk_d, wv_d, wout_d, b_d, out_d)

    nc.compile()
    return nc


def _body(nc, tc, x_d, wqk_d, wv_d, wout_d, b_d, out_d):
    from contextlib import ExitStack

    ctx = ExitStack()
    with ctx:
        const = ctx.enter_context(tc.tile_pool(name="const", bufs=1))
        qkp = ctx.enter_context(tc.tile_pool(name="qkhat", bufs=8))
        vtp = ctx.enter_context(tc.tile_pool(name="vt1", bufs=8))
        exps = ctx.enter_context(tc.tile_pool(name="exps", bufs=10))
        ohp = ctx.enter_context(tc.tile_pool(name="outh", bufs=4))
        yp = ctx.enter_context(tc.tile_pool(name="y", bufs=2))
        scr = ctx.enter_context(tc.tile_pool(name="scr", bufs=2))
        stat = ctx.enter_context(tc.tile_pool(name="stat", bufs=8))
        ps_s = ctx.enter_context(tc.tile_pool(name="ps_s", bufs=2, space="PSUM"))
        ps_u = ctx.enter_context(tc.tile_pool(name="ps_u", bufs=1, space="PSUM"))
        ps_b = ctx.enter_context(tc.tile_pool(name="ps_b", bufs=1, space="PSUM"))

        # ---- load inputs ----
        xb = []
        for c in range(2):
            t = const.tile([128, N], F32R, tag=f"xb{c}")
            nc.sync.dma_start(t[:], x_d[c * 128:(c + 1) * 128, :])
            xb.append(t)
        wqk = []
        for c in range(2):
            t = const.tile([128, 2 * HID], F32R, tag=f"wqk{c}")
            nc.sync.dma_start(t[:], wqk_d[c * 128:(c + 1) * 128, :])
            wqk.append(t)
        wv = []
        for c in range(2):
            t = const.tile([128, HID], F32R, tag=f"wv{c}")
            nc.sync.dma_start(t[:], wv_d[c * 128:(c + 1) * 128, :])
            wv.append(t)
        wout = []
        for c in range(4):
            t = const.tile([128, C], F32R, tag=f"wout{c}")
            nc.sync.dma_start(t[:], wout_d[c * 128:(c + 1) * 128, :])
            wout.append(t)
        bias = []
        for c in range(2):
            t = const.tile([128, 1], F32, tag=f"bias{c}")
            nc.sync.dma_start(t[:], b_d[c * 128:(c + 1) * 128, :])
            bias.append(t)
        ones_f = const.tile([1, DH], F32, tag="ones_f")
        nc.gpsimd.memset(ones_f[:], 1.0)
        ones64 = const.tile([1, DH], F32R, tag="ones")
        nc.vector.tensor_copy(ones64[:], ones_f[:])
        onescol_f = const.tile([128, HEADS], F32, tag="onescol")
        nc.gpsimd.memset(onescol_f[:], 1.0)

        def r(ap):
            return ap

        # ---- qk projection + l2 norm (8 o-chunks of 128: q = 0..3, k = 4..7)
        qkhat = []
        for oc in range(8):
            P = ps_s.tile([128, N], F32, tag="ps")
            for half in range(2):
                sl = slice(half * 512, (half + 1) * 512)
                for kc in range(2):
                    nc.tensor.matmul(
                        P[:, sl],
                        r(wqk[kc][:, oc * 128:(oc + 1) * 128]),
                        r(xb[kc][:, sl]),
                        start=(kc == 0),
                        stop=(kc == 1),
                    )
            sq = scr.tile([128, N], F32, tag="sq")
            ssq = stat.tile([128, 1], F32, tag="ssq")
            nc.scalar.activation(sq[:], P[:], AF.Square, accum_out=ssq[:])
            rn = stat.tile([128, 1], F32, tag="rn")
            nc.vector.reciprocal(rn[:], ssq[:])
            rs = stat.tile([128, 1], F32, tag="rs")
            # q chunks fold the SCALE=10: sqrt(100/ssq); k chunks: sqrt(1/ssq)
            nc.scalar.activation(rs[:], rn[:], AF.Sqrt,
                                 scale=100.0 if oc < 4 else 1.0)
            qh = qkp.tile([128, N], F32R, tag="qk")
            nc.vector.tensor_scalar_mul(qh[:], P[:], rs[:])
            qkhat.append(qh)

        # ---- vT projection: vT1[jc] (128, 8*65), per head [v_h | 1]
        vt1 = []
        for jc in range(8):
            Pv = ps_s.tile([128, HID], F32, tag="ps")
            for kc in range(2):
                nc.tensor.matmul(
                    Pv[:],
                    r(xb[kc][:, jc * 128:(jc + 1) * 128]),
                    r(wv[kc][:]),
                    start=(kc == 0),
                    stop=(kc == 1),
                )
            t = vtp.tile([128, HEADS * (DH + 1)], F32R, tag="vt")
            tv = t[:].rearrange("p (h e) -> p h e", e=DH + 1)
            nc.vector.tensor_copy(
                tv[:, :, DH:DH + 1],
                onescol_f[:].rearrange("p (h e) -> p h e", e=1),
            )
            nc.vector.tensor_copy(
                tv[:, :, 0:DH],
                Pv[:].rearrange("p (h e) -> p h e", e=DH),
            )
            vt1.append(t)

        # ---- attention per head ----
        outh = [ohp.tile([128, N], F32R, tag="oh", name=f"oh{i}") for i in range(4)]
        for h in range(8):
            qs = qkhat[h // 2]
            ks = qkhat[4 + h // 2]
            ro = (h % 2) * DH
            U = ps_u.tile([DH + 1, N], F32, tag="u")
            es = []
            for jc in range(8):
                S = ps_s.tile([128, N], F32, tag="ps")
                for half in range(2):
                    sl = slice(half * 512, (half + 1) * 512)
                    nc.tensor.matmul(
                        S[:, sl],
                        r(ks[ro:ro + DH, jc * 128:(jc + 1) * 128]),
                        r(qs[ro:ro + DH, sl]),
                    )
                e = exps.tile([128, N], F32R, tag="e")
                nc.scalar.activation(e[:], S[:], AF.Exp)
                es.append(e)
            for jc in range(8):
                for half in range(2):
                    sl = slice(half * 512, (half + 1) * 512)
                    nc.tensor.matmul(
                        U[:, sl],
                        r(vt1[jc][:, h * (DH + 1):(h + 1) * (DH + 1)]),
                        r(es[jc][:, sl]),
                        start=(jc == 0),
                        stop=(jc == 7),
                    )
            den = scr.tile([1, N], F32, tag="den")
            nc.vector.tensor_copy(den[:], U[DH:DH + 1, :])
            rec = scr.tile([1, N], F32, tag="rec")
            nc.vector.reciprocal_approx_fast(rec[:], den[:])
            rec_r = scr.tile([1, N], F32R, tag="recr")
            nc.vector.tensor_copy(rec_r[:], rec[:])
            Bp = ps_b.tile([DH, N], F32, tag="b")
            for half in range(2):
                sl = slice(half * 512, (half + 1) * 512)
                nc.tensor.matmul(Bp[:, sl], ones64[:], rec_r[:, sl])
            Bs = scr.tile([DH, N], F32, tag="bs")
            nc.vector.tensor_copy(Bs[:], Bp[:])
            nc.vector.tensor_mul(outh[h // 2][ro:ro + DH, :], U[0:DH, :], Bs[:])

        # ---- output projection ----
        for oc in range(2):
            Py = ps_s.tile([128, N], F32, tag="ps")
            for half in range(2):
                sl = slice(half * 512, (half + 1) * 512)
                for kc in range(4):
                    nc.tensor.matmul(
                        Py[:, sl],
                        r(wout[kc][:, oc * 128:(oc + 1) * 128]),
                        r(outh[kc][:, sl]),
                        start=(kc == 0),
                        stop=(kc == 3),
                    )
            yt = yp.tile([128, N], F32, tag="y")
            nc.vector.tensor_scalar_add(yt[:], Py[:], bias[oc][:])
            nc.sync.dma_start(out_d[oc * 128:(oc + 1) * 128, :], yt[:])


def _get_compiled():
    if "nc" not in _cache:
        _cache["nc"] = _build()
    return _cache["nc"]


def kernel(x, w_qkv, w_out, b_out, **kw):
    nc = _get_compiled()
    x = np.asarray(x, dtype=np.float32)
    w_qkv = np.asarray(w_qkv, dtype=np.float32)
    w_out = np.asarray(w_out, dtype=np.float32)
    b_out = np.asarray(b_out, dtype=np.float32)

    xs = np.ascontiguousarray(x.reshape(B, C, N))
    w_qkT = np.ascontiguousarray(w_qkv[:2 * HID].T)       # (256, 1024)
    w_vT = np.ascontiguousarray(w_qkv[2 * HID:].T)        # (256, 512)
    w_outT = np.ascontiguousarray(w_out.T)                # (512, 256)
    bb = np.ascontiguousarray(b_out.reshape(C, 1))

    in_maps = [
        {"x": xs[i], "w_qkT": w_qkT, "w_vT": w_vT, "w_outT": w_outT, "b_out": bb}
        for i in range(NCORES)
    ]
    res = run_bass_kernel_spmd(nc, in_maps, list(range(NCORES)))
    y = np.stack([res.results[i]["out"] for i in range(NCORES)])
    return y.reshape(B, C, 32, 32)


# revision 7
# speedup vs baseline: 1.2511x; 1.2511x over previous
"""Multi-head attention kernel for TRN2, 8 NeuronCores.

Problem: x (8, 256, 32, 32); qkv = w_qkv @ x_flat per batch; q, k l2-normalized
over the token axis; sim = 10 * q^T k; softmax over keys; out = attn @ v^T;
y = w_out @ out_hidden + b_out.

Sharding: pure data-parallel — batch 8 across 8 cores, one batch each.
No collectives. Weights replicated (transposed host-side to feed the PE
stationary operand directly).

Layout choices (per core, one batch):
  - qk proj computed as (o=1024, n=1024): lhsT = w_qkv[:1024].T chunks,
    rhs = x chunks. l2 norm over n = free-axis reduce (per-partition);
    norm factors stay f32, normalized q/k stored bf16 (SCALE folded into q).
  - v projected TRANSPOSED: vT (n=1024, hid=512) via lhsT = x, rhs = w_v.T.
    Stored interleaved with a ones column per head -> [v_h | 1] (65 cols/head)
    so the attention AV matmul's 65th output row is the softmax denominator.
  - S^T = k_h^T q_h per head: (j=1024, i=1024), softmax over PARTITION axis j:
    values are bounded (|S| < 1), so exp needs no max subtraction; denominator
    comes from the ones row. exp runs on ScalarE straight out of PSUM.
  - U = [v|1] @ expS^T accumulated over j chunks in PSUM (double-buffered
    across heads); normalization: recip of row 64 (DVE approx from SBUF),
    partition_broadcast on GpSimd, elementwise mul into out_hidden (bf16).
  - y = w_out.T-chunks @ out_hidden + b_out (f32), DMA out.

All matmul operands are bf16 (fast weight loads, 1 cycle/row streaming);
PSUM accumulation is f32; softmax stats and the final output stay f32.
End-to-end precision ~4e-3 relative.
"""

import numpy as np
import ml_dtypes

import concourse.bass as bass
import concourse.mybir as mybir
import concourse.tile as tile
from concourse import bacc
from concourse.bass_utils import run_bass_kernel_spmd

F32 = mybir.dt.float32
BF16 = mybir.dt.bfloat16
AF = mybir.ActivationFunctionType

B = 8          # batch (one per core)
C = 256        # input channels
N = 1024       # tokens (32*32)
HID = 512      # heads * dim_head
HEADS = 8
DH = 64
NCORES = 8

_cache = {}


def _build():
    nc = bacc.Bacc("TRN2", target_bir_lowering=False, debug=False)

    x_d = nc.dram_tensor("x", [C, N], BF16, kind="ExternalInput")
    wqk_d = nc.dram_tensor("w_qkT", [C, 2 * HID], BF16, kind="ExternalInput")
    wv_d = nc.dram_tensor("w_vT", [C, HID], BF16, kind="ExternalInput")
    wout_d = nc.dram_tensor("w_outT", [HID, C], BF16, kind="ExternalInput")
    b_d = nc.dram_tensor("b_out", [C, 1], F32, kind="ExternalInput")
    out_d = nc.dram_tensor("out", [C, N], F32, kind="ExternalOutput")

    with tile.TileContext(nc) as tc:
        _body(nc, tc, x_d, wqk_d, wv_d, wout_d, b_d, out_d)

    nc.compile()
    return nc


def _body(nc, tc, x_d, wqk_d, wv_d, wout_d, b_d, out_d):
    from contextlib import ExitStack

    ctx = ExitStack()
    with ctx:
        const = ctx.enter_context(tc.tile_pool(name="const", bufs=1))
        qkp = ctx.enter_context(tc.tile_pool(name="qkhat", bufs=8))
        vtp = ctx.enter_context(tc.tile_pool(name="vt1", bufs=8))
        exps = ctx.enter_context(tc.tile_pool(name="exps", bufs=12))
        ohp = ctx.enter_context(tc.tile_pool(name="outh", bufs=4))
        yp = ctx.enter_context(tc.tile_pool(name="y", bufs=2))
        scr = ctx.enter_context(tc.tile_pool(name="scr", bufs=3))
        stat = ctx.enter_context(tc.tile_pool(name="stat", bufs=8))
        ps_s = ctx.enter_context(tc.tile_pool(name="ps_s", bufs=2, space="PSUM"))
        ps_u = ctx.enter_context(tc.tile_pool(name="ps_u", bufs=2, space="PSUM"))

        # ---- load inputs ----
        xb = []
        for c in range(2):
            t = const.tile([128, N], BF16, tag=f"xb{c}")
            nc.sync.dma_start(t[:], x_d[c * 128:(c + 1) * 128, :])
            xb.append(t)
        wqk = []
        for c in range(2):
            t = const.tile([128, 2 * HID], BF16, tag=f"wqk{c}")
            nc.sync.dma_start(t[:], wqk_d[c * 128:(c + 1) * 128, :])
            wqk.append(t)
        wv = []
        for c in range(2):
            t = const.tile([128, HID], BF16, tag=f"wv{c}")
            nc.sync.dma_start(t[:], wv_d[c * 128:(c + 1) * 128, :])
            wv.append(t)
        wout = []
        for c in range(4):
            t = const.tile([128, C], BF16, tag=f"wout{c}")
            nc.sync.dma_start(t[:], wout_d[c * 128:(c + 1) * 128, :])
            wout.append(t)
        bias = []
        for c in range(2):
            t = const.tile([128, 1], F32, tag=f"bias{c}")
            nc.sync.dma_start(t[:], b_d[c * 128:(c + 1) * 128, :])
            bias.append(t)
        onescol_f = const.tile([128, HEADS], F32, tag="onescol")
        nc.gpsimd.memset(onescol_f[:], 1.0)

        # ---- qk projection + l2 norm (8 o-chunks of 128: q = 0..3, k = 4..7)
        qkhat = []
        for oc in range(8):
            P = ps_s.tile([128, N], F32, tag="ps")
            for half in range(2):
                sl = slice(half * 512, (half + 1) * 512)
                for kc in range(2):
                    nc.tensor.matmul(
                        P[:, sl],
                        wqk[kc][:, oc * 128:(oc + 1) * 128],
                        xb[kc][:, sl],
                        start=(kc == 0),
                        stop=(kc == 1),
                    )
            sq = scr.tile([128, N], F32, tag="sq")
            ssq = stat.tile([128, 1], F32, tag="ssq")
            nc.scalar.activation(sq[:], P[:], AF.Square, accum_out=ssq[:])
            rn = stat.tile([128, 1], F32, tag="rn")
            nc.vector.reciprocal(rn[:], ssq[:])
            rs = stat.tile([128, 1], F32, tag="rs")
            # q chunks fold the SCALE=10: sqrt(100/ssq); k chunks: sqrt(1/ssq)
            nc.scalar.activation(rs[:], rn[:], AF.Sqrt,
                                 scale=100.0 if oc < 4 else 1.0)
            qh = qkp.tile([128, N], BF16, tag="qk")
            nc.vector.tensor_scalar_mul(qh[:], P[:], rs[:])
            qkhat.append(qh)

        # ---- vT projection: vT1[jc] (128, 8*65), per head [v_h | 1]
        vt1 = []
        for jc in range(8):
            Pv = ps_s.tile([128, HID], F32, tag="ps")
            for kc in range(2):
                nc.tensor.matmul(
                    Pv[:],
                    xb[kc][:, jc * 128:(jc + 1) * 128],
                    wv[kc][:],
                    start=(kc == 0),
                    stop=(kc == 1),
                )
            t = vtp.tile([128, HEADS * (DH + 1)], BF16, tag="vt")
            tv = t[:].rearrange("p (h e) -> p h e", e=DH + 1)
            nc.vector.tensor_copy(
                tv[:, :, DH:DH + 1],
                onescol_f[:].rearrange("p (h e) -> p h e", e=1),
            )
            nc.vector.tensor_copy(
                tv[:, :, 0:DH],
                Pv[:].rearrange("p (h e) -> p h e", e=DH),
            )
            vt1.append(t)

        # ---- attention per head, tails deferred one head for PE density ----
        outh = [ohp.tile([128, N], BF16, tag="oh", name=f"oh{i}") for i in range(4)]

        def head_tail(h, U):
            ro = (h % 2) * DH
            den = scr.tile([1, N], F32, tag="den")
            nc.vector.tensor_copy(den[:], U[DH:DH + 1, :])
            rec = scr.tile([1, N], F32, tag="rec")
            nc.vector.reciprocal_approx_fast(rec[:], den[:])
            Bs = scr.tile([DH, N], F32, tag="bs")
            nc.gpsimd.partition_broadcast(Bs[:], rec[:], channels=DH)
            nc.vector.tensor_mul(outh[h // 2][ro:ro + DH, :], U[0:DH, :], Bs[:])

        pending = None  # (h, U) of previous head
        for h in range(8):
            qs = qkhat[h // 2]
            ks = qkhat[4 + h // 2]
            ro = (h % 2) * DH
            U = ps_u.tile([DH + 1, N], F32, tag="u")
            es = []
            for jc in range(8):
                S = ps_s.tile([128, N], F32, tag="ps")
                for half in range(2):
                    sl = slice(half * 512, (half + 1) * 512)
                    nc.tensor.matmul(
                        S[:, sl],
                        ks[ro:ro + DH, jc * 128:(jc + 1) * 128],
                        qs[ro:ro + DH, sl],
                    )
                e = exps.tile([128, N], BF16, tag="e")
                nc.scalar.activation(e[:], S[:], AF.Exp)
                es.append(e)
            for jc in range(8):
                for half in range(2):
                    sl = slice(half * 512, (half + 1) * 512)
                    nc.tensor.matmul(
                        U[:, sl],
                        vt1[jc][:, h * (DH + 1):(h + 1) * (DH + 1)],
                        es[jc][:, sl],
                        start=(jc == 0),
                        stop=(jc == 7),
                    )
            if pending is not None:
                head_tail(*pending)
            pending = (h, U)
        head_tail(*pending)

        # ---- output projection ----
        for oc in range(2):
            Py = ps_s.tile([128, N], F32, tag="ps")
            for half in range(2):
                sl = slice(half * 512, (half + 1) * 512)
                for kc in range(4):
                    nc.tensor.matmul(
                        Py[:, sl],
                        wout[kc][:, oc * 128:(oc + 1) * 128],
                        outh[kc][:, sl],
                        start=(kc == 0),
                        stop=(kc == 3),
                    )
            yt = yp.tile([128, N], F32, tag="y")
            nc.vector.tensor_scalar_add(yt[:], Py[:], bias[oc][:])
            nc.sync.dma_start(out_d[oc * 128:(oc + 1) * 128, :], yt[:])


def _get_compiled():
    if "nc" not in _cache:
        _cache["nc"] = _build()
    return _cache["nc"]


def _prep(x, w_qkv, w_out, b_out):
    bfc = lambda a: np.ascontiguousarray(a, dtype=ml_dtypes.bfloat16)
    xs = np.ascontiguousarray(x.reshape(B, C, N))
    return {
        "xs": xs.astype(ml_dtypes.bfloat16),
        "w_qkT": bfc(w_qkv[:2 * HID].T),
        "w_vT": bfc(w_qkv[2 * HID:].T),
        "w_outT": bfc(w_out.T),
        "b_out": np.ascontiguousarray(b_out.reshape(C, 1), dtype=np.float32),
    }


def kernel(x, w_qkv, w_out, b_out, **kw):
    nc = _get_compiled()
    x = np.asarray(x, dtype=np.float32)
    w_qkv = np.asarray(w_qkv, dtype=np.float32)
    w_out = np.asarray(w_out, dtype=np.float32)
    b_out = np.asarray(b_out, dtype=np.float32)

    p = _prep(x, w_qkv, w_out, b_out)
    in_maps = [
        {"x": p["xs"][i], "w_qkT": p["w_qkT"], "w_vT": p["w_vT"],
         "w_outT": p["w_outT"], "b_out": p["b_out"]}
        for i in range(NCORES)
    ]
    res = run_bass_kernel_spmd(nc, in_maps, list(range(NCORES)))
    y = np.stack([res.results[i]["out"] for i in range(NCORES)])
    return y.reshape(B, C, 32, 32)


# revision 9
# speedup vs baseline: 1.3735x; 1.0978x over previous
"""Multi-head attention kernel for TRN2, 8 NeuronCores.

Problem: x (8, 256, 32, 32); qkv = w_qkv @ x_flat per batch; q, k l2-normalized
over the token axis; sim = 10 * q^T k; softmax over keys; out = attn @ v^T;
y = w_out @ out_hidden + b_out.

Sharding: pure data-parallel — batch 8 across 8 cores, one batch each.
No collectives. Weights replicated (transposed host-side to feed the PE
stationary operand directly).

Layout choices (per core, one batch):
  - qk proj computed as (o=1024, n=1024): lhsT = w_qkv[:1024].T chunks,
    rhs = x chunks. l2 norm over n = free-axis reduce (per-partition);
    norm factors stay f32, normalized q/k stored bf16 (SCALE folded into q).
  - v projected TRANSPOSED: vT (n=1024, hid=512) via lhsT = x, rhs = w_v.T.
    Stored interleaved with a ones column per head -> [v_h | 1] (65 cols/head)
    so the attention AV matmul's 65th output row is the softmax denominator.
  - S^T = k_h^T q_h per head: (j=1024, i=1024), softmax over PARTITION axis j:
    values are bounded (|S| < 1), so exp needs no max subtraction; denominator
    comes from the ones row. exp runs on ScalarE straight out of PSUM.
  - U = [v|1] @ expS^T accumulated over j chunks in PSUM (double-buffered
    across heads); normalization: recip of row 64 (DVE approx from SBUF),
    partition_broadcast on GpSimd, elementwise mul into out_hidden (bf16).
  - y = w_out.T-chunks @ out_hidden + b_out (f32), DMA out.

All matmul operands are bf16 (fast weight loads, 1 cycle/row streaming);
PSUM accumulation is f32; softmax stats and the final output stay f32.
End-to-end precision ~4e-3 relative.
"""

import numpy as np
import ml_dtypes

import concourse.bass as bass
import concourse.mybir as mybir
import concourse.tile as tile
from concourse import bacc
from concourse.bass_utils import run_bass_kernel_spmd
F32 = mybir.dt.float32
BF16 = mybir.dt.bfloat16
AF = mybir.ActivationFunctionType

B = 8          # batch (one per core)
C = 256        # input channels
N = 1024       # tokens (32*32)
HID = 512      # heads * dim_head
HEADS = 8
DH = 64
NCORES = 8
XW_COLS = 6144

_cache = {}


def _build():
    nc = bacc.Bacc("TRN2", target_bir_lowering=False, debug=False)

    xw_d = nc.dram_tensor("xw", [128, XW_COLS], BF16, kind="ExternalInput")
    b_d = nc.dram_tensor("b_out", [C, 1], F32, kind="ExternalInput")
    out_d = nc.dram_tensor("out", [C, N], F32, kind="ExternalOutput")

    with tile.TileContext(nc) as tc:
        _body(nc, tc, xw_d, b_d, out_d)

    nc.compile()
    return nc


def _body(nc, tc, xw_d, b_d, out_d):
    from contextlib import ExitStack

    ctx = ExitStack()
    with ctx:
        const = ctx.enter_context(tc.tile_pool(name="const", bufs=1))
        qkp = ctx.enter_context(tc.tile_pool(name="qkhat", bufs=8))
        vtp = ctx.enter_context(tc.tile_pool(name="vt1", bufs=8))
        exps = ctx.enter_context(tc.tile_pool(name="exps", bufs=12))
        ohp = ctx.enter_context(tc.tile_pool(name="outh", bufs=4))
        yp = ctx.enter_context(tc.tile_pool(name="y", bufs=2))
        scr = ctx.enter_context(tc.tile_pool(name="scr", bufs=3))
        stat = ctx.enter_context(tc.tile_pool(name="stat", bufs=8))
        ps_s = ctx.enter_context(tc.tile_pool(name="ps_s", bufs=2, space="PSUM"))
        ps_u = ctx.enter_context(tc.tile_pool(name="ps_u", bufs=2, space="PSUM"))

        # ---- load inputs: packed [xb0|wqk0|xb1|wqk1|wv0|wv1|wout0..3],
        # critical half (kc=0) on the sync queue, rest on gpsimd queue.
        big = const.tile([128, XW_COLS], BF16, tag="big")
        nc.sync.dma_start(big[:, 0:2048], xw_d[:, 0:2048])
        nc.gpsimd.dma_start(big[:, 2048:4096], xw_d[:, 2048:4096])
        nc.sync.dma_start(big[:, 4096:XW_COLS], xw_d[:, 4096:XW_COLS])
        xb = [big[:, 0:1024], big[:, 2048:3072]]
        wqk = [big[:, 1024:2048], big[:, 3072:4096]]
        wv = [big[:, 4096:4608], big[:, 4608:5120]]
        wout = [big[:, 5120 + c * 256:5120 + (c + 1) * 256] for c in range(4)]
        bias = []
        for c in range(2):
            t = const.tile([128, 1], F32, tag=f"bias{c}")
            nc.gpsimd.dma_start(t[:], b_d[c * 128:(c + 1) * 128, :])
            bias.append(t)
        onescol_f = const.tile([128, HEADS], F32, tag="onescol")
        nc.gpsimd.memset(onescol_f[:], 1.0)

        # ---- qk projection + l2 norm (8 o-chunks of 128: q = 0..3, k = 4..7)
        qkhat = []
        for oc in range(8):
            P = ps_s.tile([128, N], F32, tag="ps")
            for half in range(2):
                sl = slice(half * 512, (half + 1) * 512)
                for kc in range(2):
                    nc.tensor.matmul(
                        P[:, sl],
                        wqk[kc][:, oc * 128:(oc + 1) * 128],
                        xb[kc][:, sl],
                        start=(kc == 0),
                        stop=(kc == 1),
                    )
            sq = scr.tile([128, N], F32, tag="sq")
            ssq = stat.tile([128, 1], F32, tag="ssq")
            nc.scalar.activation(sq[:], P[:], AF.Square, accum_out=ssq[:])
            rn = stat.tile([128, 1], F32, tag="rn")
            nc.vector.reciprocal(rn[:], ssq[:])
            rs = stat.tile([128, 1], F32, tag="rs")
            # q chunks fold the SCALE=10: sqrt(100/ssq); k chunks: sqrt(1/ssq)
            nc.scalar.activation(rs[:], rn[:], AF.Sqrt,
                                 scale=100.0 if oc < 4 else 1.0)
            qh = qkp.tile([128, N], BF16, tag="qk")
            nc.vector.tensor_scalar_mul(qh[:], P[:], rs[:])
            qkhat.append(qh)

        # ---- vT projection: vT1[jc] (128, 8*65), per head [v_h | 1]
        vt1 = []
        for jc in range(8):
            Pv = ps_s.tile([128, HID], F32, tag="ps")
            for kc in range(2):
                nc.tensor.matmul(
                    Pv[:],
                    xb[kc][:, jc * 128:(jc + 1) * 128],
                    wv[kc],
                    start=(kc == 0),
                    stop=(kc == 1),
                )
            t = vtp.tile([128, HEADS * (DH + 1)], BF16, tag="vt")
            tv = t[:].rearrange("p (h e) -> p h e", e=DH + 1)
            nc.vector.tensor_copy(
                tv[:, :, DH:DH + 1],
                onescol_f[:].rearrange("p (h e) -> p h e", e=1),
            )
            nc.vector.tensor_copy(
                tv[:, :, 0:DH],
                Pv[:].rearrange("p (h e) -> p h e", e=DH),
            )
            vt1.append(t)

        # ---- attention per head, tails deferred one head for PE density ----
        outh = [ohp.tile([128, N], BF16, tag="oh", name=f"oh{i}") for i in range(4)]

        def head_tail(h, U):
            ro = (h % 2) * DH
            den = scr.tile([1, N], F32, tag="den")
            nc.vector.tensor_copy(den[:], U[DH:DH + 1, :])
            rec = scr.tile([1, N], F32, tag="rec")
            nc.vector.reciprocal_approx_fast(rec[:], den[:])
            Bs = scr.tile([DH, N], F32, tag="bs")
            nc.gpsimd.partition_broadcast(Bs[:], rec[:], channels=DH)
            nc.vector.tensor_mul(outh[h // 2][ro:ro + DH, :], U[0:DH, :], Bs[:])

        pending = None  # (h, U) of previous head
        for h in range(8):
            qs = qkhat[h // 2]
            ks = qkhat[4 + h // 2]
            ro = (h % 2) * DH
            U = ps_u.tile([DH + 1, N], F32, tag="u")
            es = []
            for jc in range(8):
                S = ps_s.tile([128, N], F32, tag="ps")
                for half in range(2):
                    sl = slice(half * 512, (half + 1) * 512)
                    nc.tensor.matmul(
                        S[:, sl],
                        ks[ro:ro + DH, jc * 128:(jc + 1) * 128],
                        qs[ro:ro + DH, sl],
                    )
                e = exps.tile([128, N], BF16, tag="e")
                nc.scalar.activation(e[:], S[:], AF.Exp)
                es.append(e)
            for jc in range(8):
                for half in range(2):
                    sl = slice(half * 512, (half + 1) * 512)
                    nc.tensor.matmul(
                        U[:, sl],
                        vt1[jc][:, h * (DH + 1):(h + 1) * (DH + 1)],
                        es[jc][:, sl],
                        start=(jc == 0),
                        stop=(jc == 7),
                    )
            if pending is not None:
                head_tail(*pending)
            pending = (h, U)
        head_tail(*pending)

        # ---- output projection ----
        for oc in range(2):
            Py = ps_s.tile([128, N], F32, tag="ps")
            for half in range(2):
                sl = slice(half * 512, (half + 1) * 512)
                for kc in range(4):
                    nc.tensor.matmul(
                        Py[:, sl],
                        wout[kc][:, oc * 128:(oc + 1) * 128],
                        outh[kc][:, sl],
                        start=(kc == 0),
                        stop=(kc == 3),
                    )
            yt = yp.tile([128, N], F32, tag="y")
            nc.vector.tensor_scalar_add(yt[:], Py[:], bias[oc][:])
            nc.sync.dma_start(out_d[oc * 128:(oc + 1) * 128, :], yt[:])


def _get_compiled():
    if "nc" not in _cache:
        _cache["nc"] = _build()
    return _cache["nc"]


def _prep(x, w_qkv, w_out, b_out):
    bf = ml_dtypes.bfloat16
    xs = x.reshape(B, C, N).astype(bf)              # (B, 256, 1024)
    w_qkT = w_qkv[:2 * HID].T.astype(bf)            # (256, 1024)
    w_vT = w_qkv[2 * HID:].T.astype(bf)             # (256, 512)
    w_outT = w_out.T.astype(bf)                     # (512, 256)
    xw = np.empty((B, 128, XW_COLS), dtype=bf)
    for i in range(B):
        xw[i, :, 0:1024] = xs[i, :128]
        xw[i, :, 1024:2048] = w_qkT[:128]
        xw[i, :, 2048:3072] = xs[i, 128:]
        xw[i, :, 3072:4096] = w_qkT[128:]
        xw[i, :, 4096:4608] = w_vT[:128]
        xw[i, :, 4608:5120] = w_vT[128:]
        for c in range(4):
            xw[i, :, 5120 + c * 256:5120 + (c + 1) * 256] = w_outT[c * 128:(c + 1) * 128]
    return {
        "xw": np.ascontiguousarray(xw),
        "b_out": np.ascontiguousarray(b_out.reshape(C, 1), dtype=np.float32),
    }


def kernel(x, w_qkv, w_out, b_out, **kw):
    nc = _get_compiled()
    x = np.asarray(x, dtype=np.float32)
    w_qkv = np.asarray(w_qkv, dtype=np.float32)
    w_out = np.asarray(w_out, dtype=np.float32)
    b_out = np.asarray(b_out, dtype=np.float32)

    p = _prep(x, w_qkv, w_out, b_out)
    in_maps = [
        {"xw": p["xw"][i], "b_out": p["b_out"]}
        for i in range(NCORES)
    ]
    res = run_bass_kernel_spmd(nc, in_maps, list(range(NCORES)))
    y = np.stack([res.results[i]["out"] for i in range(NCORES)])
    return y.reshape(B, C, 32, 32)


# revision 11
# speedup vs baseline: 1.3822x; 1.0064x over previous
"""Multi-head attention kernel for TRN2, 8 NeuronCores.

Problem: x (8, 256, 32, 32); qkv = w_qkv @ x_flat per batch; q, k l2-normalized
over the token axis; sim = 10 * q^T k; softmax over keys; out = attn @ v^T;
y = w_out @ out_hidden + b_out.

Sharding: pure data-parallel — batch 8 across 8 cores, one batch each.
No collectives. Weights replicated (transposed host-side to feed the PE
stationary operand directly).

Layout choices (per core, one batch):
  - qk proj computed as (o=1024, n=1024): lhsT = w_qkv[:1024].T chunks,
    rhs = x chunks. l2 norm over n = free-axis reduce (per-partition);
    norm factors stay f32, normalized q/k stored bf16 (SCALE folded into q).
  - v projected TRANSPOSED: vT (n=1024, hid=512) via lhsT = x, rhs = w_v.T.
    Stored interleaved with a ones column per head -> [v_h | 1] (65 cols/head)
    so the attention AV matmul's 65th output row is the softmax denominator.
  - S^T = k_h^T q_h per head: (j=1024, i=1024), softmax over PARTITION axis j:
    values are bounded (|S| < 1), so exp needs no max subtraction; denominator
    comes from the ones row. exp runs on ScalarE straight out of PSUM.
  - U = [v|1] @ expS^T accumulated over j chunks in PSUM (double-buffered
    across heads); normalization: recip of row 64 (DVE approx from SBUF),
    partition_broadcast on GpSimd, elementwise mul into out_hidden (bf16).
  - y = w_out.T-chunks @ out_hidden + b_out (f32), DMA out.

All matmul operands are bf16 (fast weight loads, 1 cycle/row streaming);
PSUM accumulation is f32; softmax stats and the final output stay f32.
End-to-end precision ~4e-3 relative.
"""

import numpy as np
import ml_dtypes

import concourse.bass as bass
import concourse.mybir as mybir
import concourse.tile as tile
from concourse import bacc
from concourse.bass_utils import run_bass_kernel_spmd
F32 = mybir.dt.float32
BF16 = mybir.dt.bfloat16
AF = mybir.ActivationFunctionType

B = 8          # batch (one per core)
C = 256        # input channels
N = 1024       # tokens (32*32)
HID = 512      # heads * dim_head
HEADS = 8
DH = 64
NCORES = 8
XW_COLS = 6144

_cache = {}


def _build():
    nc = bacc.Bacc("TRN2", target_bir_lowering=False, debug=False)

    xw_d = nc.dram_tensor("xw", [128, XW_COLS], BF16, kind="ExternalInput")
    b_d = nc.dram_tensor("b_out", [C, 1], F32, kind="ExternalInput")
    out_d = nc.dram_tensor("out", [C, N], F32, kind="ExternalOutput")

    with tile.TileContext(nc) as tc:
        _body(nc, tc, xw_d, b_d, out_d)

    nc.compile()
    return nc


def _body(nc, tc, xw_d, b_d, out_d):
    from contextlib import ExitStack

    ctx = ExitStack()
    with ctx:
        const = ctx.enter_context(tc.tile_pool(name="const", bufs=1))
        qkp = ctx.enter_context(tc.tile_pool(name="qkhat", bufs=8))
        vtp = ctx.enter_context(tc.tile_pool(name="vt1", bufs=8))
        exps = ctx.enter_context(tc.tile_pool(name="exps", bufs=12))
        ohp = ctx.enter_context(tc.tile_pool(name="outh", bufs=4))
        yp = ctx.enter_context(tc.tile_pool(name="y", bufs=2))
        scr = ctx.enter_context(tc.tile_pool(name="scr", bufs=3))
        stat = ctx.enter_context(tc.tile_pool(name="stat", bufs=8))
        ps_s = ctx.enter_context(tc.tile_pool(name="ps_s", bufs=2, space="PSUM"))
        ps_u = ctx.enter_context(tc.tile_pool(name="ps_u", bufs=2, space="PSUM"))

        # ---- load inputs: packed [xb0|wqk0|xb1|wqk1|wv0|wv1|wout0..3],
        # critical half (kc=0) on the sync queue, rest on gpsimd queue.
        big = const.tile([128, XW_COLS], BF16, tag="big")
        nc.sync.dma_start(big[:, 0:2048], xw_d[:, 0:2048])
        nc.gpsimd.dma_start(big[:, 2048:4096], xw_d[:, 2048:4096])
        nc.sync.dma_start(big[:, 4096:XW_COLS], xw_d[:, 4096:XW_COLS])
        xb = [big[:, 0:1024], big[:, 2048:3072]]
        wqk = [big[:, 1024:2048], big[:, 3072:4096]]
        wv = [big[:, 4096:4608], big[:, 4608:5120]]
        wout = [big[:, 5120 + c * 256:5120 + (c + 1) * 256] for c in range(4)]
        bias = []
        for c in range(2):
            t = const.tile([128, 1], F32, tag=f"bias{c}")
            nc.gpsimd.dma_start(t[:], b_d[c * 128:(c + 1) * 128, :])
            bias.append(t)
        onescol_f = const.tile([128, HEADS], F32, tag="onescol")
        nc.gpsimd.memset(onescol_f[:], 1.0)

        # ---- qk projection + l2 norm (8 o-chunks of 128: q = 0..3, k = 4..7)
        # Emission order: chunks 0 and 4 first (head 0's q/k), then the vT
        # projection, then the rest — lets attention start early. Proj PSUM
        # alternates between the ps_s and (idle during proj) ps_u pools for
        # 4 effective slots.
        qkhat = [None] * 8
        proj_order = [0, 4, 1, 5, 2, 6, 3, 7]

        def qk_chunk(oc, pool, ptag):
            P = pool.tile([128, N], F32, tag=ptag, name=f"pqk{oc}")
            for half in range(2):
                sl = slice(half * 512, (half + 1) * 512)
                for kc in range(2):
                    nc.tensor.matmul(
                        P[:, sl],
                        wqk[kc][:, oc * 128:(oc + 1) * 128],
                        xb[kc][:, sl],
                        start=(kc == 0),
                        stop=(kc == 1),
                    )
            sq = scr.tile([128, N], F32, tag="sq", name=f"sq{oc}")
            ssq = stat.tile([128, 1], F32, tag="ssq", name=f"ssq{oc}")
            nc.scalar.activation(sq[:], P[:], AF.Square, accum_out=ssq[:])
            rn = stat.tile([128, 1], F32, tag="rn", name=f"rn{oc}")
            nc.vector.reciprocal(rn[:], ssq[:])
            rs = stat.tile([128, 1], F32, tag="rs", name=f"rs{oc}")
            # q chunks fold the SCALE=10: sqrt(100/ssq); k chunks: sqrt(1/ssq)
            nc.scalar.activation(rs[:], rn[:], AF.Sqrt,
                                 scale=100.0 if oc < 4 else 1.0)
            qh = qkp.tile([128, N], BF16, tag="qk", name=f"qk{oc}")
            nc.vector.tensor_scalar_mul(qh[:], P[:], rs[:])
            qkhat[oc] = qh

        for idx, oc in enumerate(proj_order[:2]):
            qk_chunk(oc, (ps_s, ps_u)[idx % 2], ("ps", "u")[idx % 2])

        # ---- vT projection: vT1[jc] (128, 8*65), per head [v_h | 1]
        vt1 = []
        for jc in range(8):
            pool, ptag = ((ps_s, "ps"), (ps_u, "u"))[jc % 2]
            Pv = pool.tile([128, HID], F32, tag=ptag, name=f"pv{jc}")
            for kc in range(2):
                nc.tensor.matmul(
                    Pv[:],
                    xb[kc][:, jc * 128:(jc + 1) * 128],
                    wv[kc],
                    start=(kc == 0),
                    stop=(kc == 1),
                )
            t = vtp.tile([128, HEADS * (DH + 1)], BF16, tag="vt", name=f"vt{jc}")
            tv = t[:].rearrange("p (h e) -> p h e", e=DH + 1)
            nc.vector.tensor_copy(
                tv[:, :, DH:DH + 1],
                onescol_f[:].rearrange("p (h e) -> p h e", e=1),
            )
            nc.vector.tensor_copy(
                tv[:, :, 0:DH],
                Pv[:].rearrange("p (h e) -> p h e", e=DH),
            )
            vt1.append(t)

        for idx, oc in enumerate(proj_order[2:]):
            qk_chunk(oc, (ps_s, ps_u)[idx % 2], ("ps", "u")[idx % 2])

        # ---- attention per head, tails deferred one head for PE density ----
        outh = [ohp.tile([128, N], BF16, tag="oh", name=f"oh{i}") for i in range(4)]

        def head_tail(h, U):
            ro = (h % 2) * DH
            den = scr.tile([1, N], F32, tag="den", name=f"den{h}")
            nc.vector.tensor_copy(den[:], U[DH:DH + 1, :])
            rec = scr.tile([1, N], F32, tag="rec", name=f"rec{h}")
            nc.vector.reciprocal_approx_fast(rec[:], den[:])
            Bs = scr.tile([DH, N], F32, tag="bs", name=f"bs{h}")
            nc.gpsimd.partition_broadcast(Bs[:], rec[:], channels=DH)
            nc.vector.tensor_mul(outh[h // 2][ro:ro + DH, :], U[0:DH, :], Bs[:])

        pending = None  # (h, U) of previous head
        for h in range(8):
            qs = qkhat[h // 2]
            ks = qkhat[4 + h // 2]
            ro = (h % 2) * DH
            U = ps_u.tile([DH + 1, N], F32, tag="u", name=f"u{h}")
            es = []
            for jc in range(8):
                S = ps_s.tile([128, N], F32, tag="ps", name=f"s{h}_{jc}")
                for half in range(2):
                    sl = slice(half * 512, (half + 1) * 512)
                    nc.tensor.matmul(
                        S[:, sl],
                        ks[ro:ro + DH, jc * 128:(jc + 1) * 128],
                        qs[ro:ro + DH, sl],
                    )
                e = exps.tile([128, N], BF16, tag="e", name=f"e{h}_{jc}")
                nc.scalar.activation(e[:], S[:], AF.Exp)
                es.append(e)
            if pending is not None:
                head_tail(*pending)
            for jc in range(8):
                for half in range(2):
                    sl = slice(half * 512, (half + 1) * 512)
                    nc.tensor.matmul(
                        U[:, sl],
                        vt1[jc][:, h * (DH + 1):(h + 1) * (DH + 1)],
                        es[jc][:, sl],
                        start=(jc == 0),
                        stop=(jc == 7),
                    )
            pending = (h, U)
        head_tail(*pending)

        # ---- output projection ----
        for oc in range(2):
            Py = ps_s.tile([128, N], F32, tag="ps")
            for half in range(2):
                sl = slice(half * 512, (half + 1) * 512)
                for kc in range(4):
                    nc.tensor.matmul(
                        Py[:, sl],
                        wout[kc][:, oc * 128:(oc + 1) * 128],
                        outh[kc][:, sl],
                        start=(kc == 0),
                        stop=(kc == 3),
                    )
            yt = yp.tile([128, N], F32, tag="y")
            nc.vector.tensor_scalar_add(yt[:], Py[:], bias[oc][:])
            nc.sync.dma_start(out_d[oc * 128:(oc + 1) * 128, :], yt[:])


def _get_compiled():
    if "nc" not in _cache:
        _cache["nc"] = _build()
    return _cache["nc"]


def _prep(x, w_qkv, w_out, b_out):
    bf = ml_dtypes.bfloat16
    xs = x.reshape(B, C, N).astype(bf)              # (B, 256, 1024)
    w_qkT = w_qkv[:2 * HID].T.astype(bf)            # (256, 1024)
    w_vT = w_qkv[2 * HID:].T.astype(bf)             # (256, 512)
    w_outT = w_out.T.astype(bf)                     # (512, 256)
    xw = np.empty((B, 128, XW_COLS), dtype=bf)
    for i in range(B):
        xw[i, :, 0:1024] = xs[i, :128]
        xw[i, :, 1024:2048] = w_qkT[:128]
        xw[i, :, 2048:3072] = xs[i, 128:]
        xw[i, :, 3072:4096] = w_qkT[128:]
        xw[i, :, 4096:4608] = w_vT[:128]
        xw[i, :, 4608:5120] = w_vT[128:]
        for c in range(4):
            xw[i, :, 5120 + c * 256:5120 + (c + 1) * 256] = w_outT[c * 128:(c + 1) * 128]
    return {
        "xw": np.ascontiguousarray(xw),
        "b_out": np.ascontiguousarray(b_out.reshape(C, 1), dtype=np.float32),
    }


def kernel(x, w_qkv, w_out, b_out, **kw):
    nc = _get_compiled()
    x = np.asarray(x, dtype=np.float32)
    w_qkv = np.asarray(w_qkv, dtype=np.float32)
    w_out = np.asarray(w_out, dtype=np.float32)
    b_out = np.asarray(b_out, dtype=np.float32)

    p = _prep(x, w_qkv, w_out, b_out)
    in_maps = [
        {"xw": p["xw"][i], "b_out": p["b_out"]}
        for i in range(NCORES)
    ]
    res = run_bass_kernel_spmd(nc, in_maps, list(range(NCORES)))
    y = np.stack([res.results[i]["out"] for i in range(NCORES)])
    return y.reshape(B, C, 32, 32)


# revision 14
# speedup vs baseline: 1.5037x; 1.0879x over previous
"""Multi-head attention kernel for TRN2, 8 NeuronCores.

Problem: x (8, 256, 32, 32); qkv = w_qkv @ x_flat per batch; q, k l2-normalized
over the token axis; sim = 10 * q^T k; softmax over keys; out = attn @ v^T;
y = w_out @ out_hidden + b_out.

Sharding: pure data-parallel — batch 8 across 8 cores, one batch each.
No collectives. Weights replicated (transposed host-side to feed the PE
stationary operand directly).

Layout choices (per core, one batch):
  - qk proj computed as (o=1024, n=1024): lhsT = w_qkv[:1024].T chunks,
    rhs = x chunks. l2 norm over n = free-axis reduce (per-partition);
    norm factors stay f32, normalized q/k stored bf16 (SCALE folded into q).
  - v projected TRANSPOSED: vT (n=1024, hid=512) via lhsT = x, rhs = w_v.T.
    Stored interleaved with a ones column per head -> [v_h | 1] (65 cols/head)
    so the attention AV matmul's 65th output row is the softmax denominator.
  - S^T = k_h^T q_h per head: (j=1024, i=1024), softmax over PARTITION axis j:
    values are bounded (|S| < 1), so exp needs no max subtraction; denominator
    comes from the ones row. exp runs on ScalarE straight out of PSUM.
  - U = [v|1] @ expS^T accumulated over j chunks in PSUM (double-buffered
    across heads); normalization: recip of row 64 (DVE approx from SBUF),
    partition_broadcast on GpSimd, elementwise mul into out_hidden (bf16).
  - y = w_out.T-chunks @ out_hidden + b_out (f32), DMA out.

All matmul operands are bf16 (fast weight loads, 1 cycle/row streaming);
PSUM accumulation is f32; softmax stats and the final output stay f32.
End-to-end precision ~4e-3 relative.
"""

import numpy as np
import ml_dtypes

import concourse.bass as bass
import concourse.mybir as mybir
import concourse.tile as tile
from concourse import bacc
from concourse.bass_utils import run_bass_kernel_spmd
F32 = mybir.dt.float32
BF16 = mybir.dt.bfloat16
AF = mybir.ActivationFunctionType

B = 8          # batch (one per core)
C = 256        # input channels
N = 1024       # tokens (32*32)
HID = 512      # heads * dim_head
HEADS = 8
DH = 64
NCORES = 8
XW_COLS = 6144

_cache = {}


def _build():
    nc = bacc.Bacc("TRN2", target_bir_lowering=False, debug=False)

    xw_d = nc.dram_tensor("xw", [128, XW_COLS], BF16, kind="ExternalInput")
    b_d = nc.dram_tensor("b_out", [C, 1], F32, kind="ExternalInput")
    out_d = nc.dram_tensor("out", [C, N], F32, kind="ExternalOutput")

    with tile.TileContext(nc) as tc:
        _body(nc, tc, xw_d, b_d, out_d)

    nc.compile()
    return nc


def _body(nc, tc, xw_d, b_d, out_d):
    from contextlib import ExitStack

    ctx = ExitStack()
    with ctx:
        const = ctx.enter_context(tc.tile_pool(name="const", bufs=1))
        qkp = ctx.enter_context(tc.tile_pool(name="qkhat", bufs=8))
        vtp = ctx.enter_context(tc.tile_pool(name="vt1", bufs=8))
        exps = ctx.enter_context(tc.tile_pool(name="exps", bufs=12))
        ohp = ctx.enter_context(tc.tile_pool(name="outh", bufs=4))
        yp = ctx.enter_context(tc.tile_pool(name="y", bufs=2))
        scr = ctx.enter_context(tc.tile_pool(name="scr", bufs=3))
        stat = ctx.enter_context(tc.tile_pool(name="stat", bufs=8))
        ps_s = ctx.enter_context(tc.tile_pool(name="ps_s", bufs=2, space="PSUM"))
        ps_u = ctx.enter_context(tc.tile_pool(name="ps_u", bufs=2, space="PSUM"))

        # ---- load inputs: packed [xb0|wqk0|xb1|wqk1|wv0|wv1|wout0..3],
        # critical half (kc=0) on the sync queue, rest on gpsimd queue.
        big = const.tile([128, XW_COLS], BF16, tag="big")
        nc.sync.dma_start(big[:, 0:4096], xw_d[:, 0:4096])
        nc.gpsimd.dma_start(big[:, 4096:XW_COLS], xw_d[:, 4096:XW_COLS])
        xb = [big[:, 0:1024], big[:, 1024:2048]]
        wqk = [big[:, 2048:3072], big[:, 3072:4096]]
        wv = [big[:, 4096:4608], big[:, 4608:5120]]
        wout = [big[:, 5120 + c * 256:5120 + (c + 1) * 256] for c in range(4)]
        bias = []
        for c in range(2):
            t = const.tile([128, 1], F32, tag=f"bias{c}")
            nc.gpsimd.dma_start(t[:], b_d[c * 128:(c + 1) * 128, :])
            bias.append(t)
        onescol_f = const.tile([128, HEADS], F32, tag="onescol")
        nc.gpsimd.memset(onescol_f[:], 1.0)

        # PE warmup: junk matmuls on memset tiles ride out the NEFF prologue
        # and input-DMA window so HAM reaches 8/8 before real work arrives.
        wu_w = const.tile([128, 128], BF16, tag="wu_w")
        nc.gpsimd.memset(wu_w[:].bitcast(F32)[:, 0:64], 0.0)
        wu_r = const.tile([128, 512], BF16, tag="wu_r")
        nc.gpsimd.memset(wu_r[:].bitcast(F32)[:, 0:256], 0.0)
        wu_p = ps_s.tile([128, 512], F32, tag="ps", name="wu_p")
        for _ in range(10):
            nc.tensor.matmul(wu_p[:], wu_w[:], wu_r[:])

        # ---- qk projection + l2 norm (8 o-chunks of 128: q = 0..3, k = 4..7)
        # Emission order: chunks 0 and 4 first (head 0's q/k), then the vT
        # projection, then the rest — lets attention start early. Proj PSUM
        # alternates between the ps_s and (idle during proj) ps_u pools for
        # 4 effective slots.
        qkhat = [None] * 8

        def proj_mms(oc, pool, ptag):
            P = pool.tile([128, N], F32, tag=ptag, name=f"pqk{oc}")
            for half in range(2):
                sl = slice(half * 512, (half + 1) * 512)
                for kc in range(2):
                    nc.tensor.matmul(
                        P[:, sl],
                        wqk[kc][:, oc * 128:(oc + 1) * 128],
                        xb[kc][:, sl],
                        start=(kc == 0),
                        stop=(kc == 1),
                    )
            return P

        def qk_pair(qc):
            # q chunk qc (0..3) and its k partner qc+4; the q-side norm factor
            # (with SCALE=10 folded) multiplies the K side: S = q_raw^T k_tilde.
            Pq = proj_mms(qc, ps_s, "ps")
            Pk = proj_mms(qc + 4, ps_u, "u")
            ssq = stat.tile([128, 1], F32, tag="ssq", name=f"ssq{qc}")
            sq = scr.tile([128, N], F32, tag="sq", name=f"sq{qc}")
            nc.scalar.activation(sq[:], Pq[:], AF.Square, accum_out=ssq[:])
            ssk = stat.tile([128, 1], F32, tag="ssk", name=f"ssk{qc}")
            sk = scr.tile([128, N], F32, tag="sq", name=f"sk{qc}")
            nc.scalar.activation(sk[:], Pk[:], AF.Square, accum_out=ssk[:])
            qh = qkp.tile([128, N], BF16, tag="qk", name=f"qk{qc}")
            nc.vector.tensor_copy(qh[:], Pq[:])
            rnq = stat.tile([128, 1], F32, tag="rnq", name=f"rnq{qc}")
            nc.vector.reciprocal(rnq[:], ssq[:])
            rsq = stat.tile([128, 1], F32, tag="rsq", name=f"rsq{qc}")
            nc.scalar.activation(rsq[:], rnq[:], AF.Sqrt, scale=100.0)
            rnk = stat.tile([128, 1], F32, tag="rnk", name=f"rnk{qc}")
            nc.vector.reciprocal(rnk[:], ssk[:])
            rsk = stat.tile([128, 1], F32, tag="rsk", name=f"rsk{qc}")
            nc.scalar.activation(rsk[:], rnk[:], AF.Sqrt)
            rqk = stat.tile([128, 1], F32, tag="rqk", name=f"rqk{qc}")
            nc.vector.tensor_mul(rqk[:], rsq[:], rsk[:])
            kh = qkp.tile([128, N], BF16, tag="qk", name=f"kh{qc}")
            nc.vector.tensor_scalar_mul(kh[:], Pk[:], rqk[:])
            qkhat[qc] = qh
            qkhat[qc + 4] = kh

        qk_pair(0)

        # ---- vT projection: vT1[jc] (128, 8*65), per head [v_h | 1]
        vt1 = []
        for jc in range(8):
            pool, ptag = ((ps_s, "ps"), (ps_u, "u"))[jc % 2]
            Pv = pool.tile([128, HID], F32, tag=ptag, name=f"pv{jc}")
            for kc in range(2):
                nc.tensor.matmul(
                    Pv[:],
                    xb[kc][:, jc * 128:(jc + 1) * 128],
                    wv[kc],
                    start=(kc == 0),
                    stop=(kc == 1),
                )
            t = vtp.tile([128, HEADS * (DH + 1)], BF16, tag="vt", name=f"vt{jc}")
            tv = t[:].rearrange("p (h e) -> p h e", e=DH + 1)
            nc.vector.tensor_copy(
                tv[:, :, DH:DH + 1],
                onescol_f[:].rearrange("p (h e) -> p h e", e=1),
            )
            nc.vector.tensor_copy(
                tv[:, :, 0:DH],
                Pv[:].rearrange("p (h e) -> p h e", e=DH),
            )
            vt1.append(t)

        for qc in range(1, 4):
            qk_pair(qc)

        # ---- attention per head, tails deferred one head for PE density ----
        outh = [ohp.tile([128, N], BF16, tag="oh", name=f"oh{i}") for i in range(4)]

        def head_tail(h, U):
            ro = (h % 2) * DH
            den = scr.tile([1, N], F32, tag="den", name=f"den{h}")
            nc.vector.tensor_copy(den[:], U[DH:DH + 1, :])
            rec = scr.tile([1, N], F32, tag="rec", name=f"rec{h}")
            nc.vector.reciprocal_approx_fast(rec[:], den[:])
            Bs = scr.tile([DH, N], F32, tag="bs", name=f"bs{h}")
            nc.gpsimd.partition_broadcast(Bs[:], rec[:], channels=DH)
            nc.vector.tensor_mul(outh[h // 2][ro:ro + DH, :], U[0:DH, :], Bs[:])

        pending = None  # (h, U) of previous head
        for h in range(8):
            qs = qkhat[h // 2]
            ks = qkhat[4 + h // 2]
            ro = (h % 2) * DH
            U = ps_u.tile([DH + 1, N], F32, tag="u", name=f"u{h}")
            es = []
            for jc in range(8):
                S = ps_s.tile([128, N], F32, tag="ps", name=f"s{h}_{jc}")
                for half in range(2):
                    sl = slice(half * 512, (half + 1) * 512)
                    nc.tensor.matmul(
                        S[:, sl],
                        ks[ro:ro + DH, jc * 128:(jc + 1) * 128],
                        qs[ro:ro + DH, sl],
                    )
                e = exps.tile([128, N], BF16, tag="e", name=f"e{h}_{jc}")
                nc.scalar.activation(e[:], S[:], AF.Exp)
                es.append(e)
            if pending is not None:
                head_tail(*pending)
            for jc in range(8):
                for half in range(2):
                    sl = slice(half * 512, (half + 1) * 512)
                    nc.tensor.matmul(
                        U[:, sl],
                        vt1[jc][:, h * (DH + 1):(h + 1) * (DH + 1)],
                        es[jc][:, sl],
                        start=(jc == 0),
                        stop=(jc == 7),
                    )
            pending = (h, U)
        head_tail(*pending)

        # ---- output projection ----
        for oc in range(2):
            Py = ps_s.tile([128, N], F32, tag="ps")
            for half in range(2):
                sl = slice(half * 512, (half + 1) * 512)
                for kc in range(4):
                    nc.tensor.matmul(
                        Py[:, sl],
                        wout[kc][:, oc * 128:(oc + 1) * 128],
                        outh[kc][:, sl],
                        start=(kc == 0),
                        stop=(kc == 3),
                    )
            yt = yp.tile([128, N], F32, tag="y")
            nc.scalar.activation(yt[:], Py[:], AF.Identity, bias=bias[oc][:])
            nc.sync.dma_start(out_d[oc * 128:(oc + 1) * 128, :], yt[:])


def _get_compiled():
    if "nc" not in _cache:
        _cache["nc"] = _build()
    return _cache["nc"]


def _prep(x, w_qkv, w_out, b_out):
    bf = ml_dtypes.bfloat16
    xs = x.reshape(B, C, N).astype(bf)              # (B, 256, 1024)
    w_qkT = w_qkv[:2 * HID].T.astype(bf)            # (256, 1024)
    w_vT = w_qkv[2 * HID:].T.astype(bf)             # (256, 512)
    w_outT = w_out.T.astype(bf)                     # (512, 256)
    xw = np.empty((B, 128, XW_COLS), dtype=bf)
    for i in range(B):
        xw[i, :, 0:1024] = xs[i, :128]
        xw[i, :, 1024:2048] = xs[i, 128:]
        xw[i, :, 2048:3072] = w_qkT[:128]
        xw[i, :, 3072:4096] = w_qkT[128:]
        xw[i, :, 4096:4608] = w_vT[:128]
        xw[i, :, 4608:5120] = w_vT[128:]
        for c in range(4):
            xw[i, :, 5120 + c * 256:5120 + (c + 1) * 256] = w_outT[c * 128:(c + 1) * 128]
    return {
        "xw": np.ascontiguousarray(xw),
        "b_out": np.ascontiguousarray(b_out.reshape(C, 1), dtype=np.float32),
    }


def kernel(x, w_qkv, w_out, b_out, **kw):
    nc = _get_compiled()
    x = np.asarray(x, dtype=np.float32)
    w_qkv = np.asarray(w_qkv, dtype=np.float32)
    w_out = np.asarray(w_out, dtype=np.float32)
    b_out = np.asarray(b_out, dtype=np.float32)

    p = _prep(x, w_qkv, w_out, b_out)
    in_maps = [
        {"xw": p["xw"][i], "b_out": p["b_out"]}
        for i in range(NCORES)
    ]
    res = run_bass_kernel_spmd(nc, in_maps, list(range(NCORES)))
    y = np.stack([res.results[i]["out"] for i in range(NCORES)])
    return y.reshape(B, C, 32, 32)
